# revision 26
# baseline (speedup 1.0000x reference)
"""DualAxisAggAttn Trainium2 kernel v3 (8-core data-parallel over batch).

Per axis A in {W, H} (x: [B,C,H,W], O = 1+2C):
  qkv = conv1x1(x); s = softmax(q, axis=A); ctx = sum_A(k*s)
  out = x + sigmoid(v)*ctx; y = SiLU(BN(dw3x3(out))); x' = out + y

v3 strategy per core (2 images, channel halves on partitions, bf16):
  - packed row-padded layout P [128, 2+82*80] bf16 per (img, half): 80-elem
    zero rows above/below the 80x80 data + 1 guard elem each end. x is loaded
    straight into P via a casting Pool DMA (f32->bf16, contiguous).
  - q replicated over partitions via rank-1 lhsT (wq (x) ones); ACT Exp evicts
    e = exp(q) broadcast (softmax shift dropped; |q| small). ctx =
    (Wk @ (u/Z)) * 0.5 + bk/2, u/Z = bf16 fold-tree + reduce on DVE.
  - sigmoid via tanh trick; t2 = (th+1)*ctx2 on DVE; out = cur + t2 done
    IN-PLACE on P by a Pool accumulate-DMA (P += t2), so v/q/xs read the same
    buffer dw later consumes; no separate out pass.
  - dw 3x3 = 9 diag bf16 matmuls on flat shifted views of P; row-wrap
    contamination at the two edge columns is cancelled by 6 tiny negated-diag
    fixup matmuls into the same PSUM group (zero rows absorb dy overreach).
  - finals out+y: packed tensor_tensor (bf16 2x for stage W, writing the next
    stage's P directly; f32 to DRAM for stage H).
  - phases of both images interleaved for cross-image engine pipelining.
"""

import numpy as np

B, C, H, W = 16, 256, 80, 80
HW = H * W
NCORES = 8
BPC = B // NCORES
TS = 2 + W * (H + 2)   # 6562
D0 = 1 + W             # 81: flat offset of data row 0
RCH = 6
NCH = (H + RCH - 1) // RCH   # 14
NPAIR = (NCH + 1) // 2       # 7
XCH = 800
QF = 480
BN_EPS = 1e-5
FOLD_RED = True
TAPS = [(dy, dx) for dy in (-1, 0, 1) for dx in (-1, 0, 1)]
FP8_STAGES = ("W",)  # stages whose depthwise conv runs in fp8 (weight hi+lo split)
FIXUPS = [(dy, dx) for dy in (-1, 0, 1) for dx in (-1, 1)]

_CACHE = {}


def _build(n_img=BPC):
    import concourse.bass as bass
    import concourse.bacc as bacc
    import concourse.mybir as mybir
    import concourse.tile as tile

    f32 = mybir.dt.float32
    bf16 = mybir.dt.bfloat16
    f8 = mybir.dt.float8e4
    Alu = mybir.AluOpType
    Act = mybir.ActivationFunctionType
    X = mybir.AxisListType.X

    nc = bacc.Bacc("TRN2", target_bir_lowering=False, debug=False)

    xd = nc.declare_dram_parameter("x", [n_img, C, HW], f32, isOutput=False)
    prm = {}
    for st in ("W", "H"):
        prm[st] = {
            "wvT": nc.declare_dram_parameter(f"wvT_{st}", [C, C], bf16, isOutput=False),
            "wkT": nc.declare_dram_parameter(f"wkT_{st}", [C, C], bf16, isOutput=False),
            "wqrT": nc.declare_dram_parameter(f"wqrT_{st}", [C, 128], bf16, isOutput=False),
            "bv2": nc.declare_dram_parameter(f"bv2_{st}", [C], f32, isOutput=False),
            "bk2": nc.declare_dram_parameter(f"bk2_{st}", [C], f32, isOutput=False),
            "bns": nc.declare_dram_parameter(f"bns_{st}", [C], f32, isOutput=False),
            "bnsh": nc.declare_dram_parameter(f"bnsh_{st}", [C], f32, isOutput=False),
            "dws": nc.declare_dram_parameter(f"dws_{st}", [2, 9, 128, 128], bf16, isOutput=False),
            "dwsn": nc.declare_dram_parameter(f"dwsn_{st}", [2, 6, 128, 128], bf16, isOutput=False),
            "dws8": nc.declare_dram_parameter(f"dws8_{st}", [2, 9, 128, 256], f8, isOutput=False),
            "dwsn8": nc.declare_dram_parameter(f"dwsn8_{st}", [2, 6, 128, 256], f8, isOutput=False),
        }
    outd = nc.declare_dram_parameter("out", [n_img, C, HW], f32, isOutput=True)

    with tile.TileContext(nc) as tc:
        with (
            tc.tile_pool(name="wgt", bufs=1) as wgt,
            tc.tile_pool(name="P", bufs=3) as Pp,
            tc.tile_pool(name="big", bufs=6) as bigp,
            tc.tile_pool(name="th", bufs=2) as thp,
            tc.tile_pool(name="P8", bufs=1) as P8p,
            tc.tile_pool(name="ys", bufs=2) as ysp,
            tc.tile_pool(name="och", bufs=2) as ochp,
            tc.tile_pool(name="scr", bufs=1) as scrp,
            tc.tile_pool(name="sml", bufs=1) as sml,
            tc.tile_pool(name="psq", bufs=3, space="PSUM") as psq,
            tc.tile_pool(name="psv", bufs=2, space="PSUM") as psv,
            tc.tile_pool(name="psdw", bufs=3, space="PSUM") as psdw,
        ):
            lp = lambda: nc.allow_low_precision(reason="bf16 within tolerance")

            # ---------------- weights ----------------
            SW = {}
            for st in ("W", "H"):
                p = prm[st]
                lv, lk, lqr = [], [], []
                for kt in range(2):
                    sl = slice(kt * 128, (kt + 1) * 128)
                    tv = wgt.tile([128, C], bf16, tag=f"lv{st}{kt}", name=f"lv{st}{kt}")
                    nc.sync.dma_start(out=tv[:], in_=p["wvT"][sl, :])
                    lv.append(tv)
                    tk = wgt.tile([128, C], bf16, tag=f"lk{st}{kt}", name=f"lk{st}{kt}")
                    nc.sync.dma_start(out=tk[:], in_=p["wkT"][sl, :])
                    lk.append(tk)
                    tq = wgt.tile([128, 128], bf16, tag=f"lq{st}{kt}", name=f"lq{st}{kt}")
                    nc.sync.dma_start(out=tq[:], in_=p["wqrT"][sl, :])
                    lqr.append(tq)
                bias = {}
                for nm in ("bv2", "bk2", "bns", "bnsh"):
                    bias[nm] = []
                    for mt in range(2):
                        sl = slice(mt * 128, (mt + 1) * 128)
                        t = wgt.tile([128, 1], f32, tag=f"{nm}{st}{mt}", name=f"{nm}{st}{mt}")
                        nc.sync.dma_start(out=t[:], in_=p[nm][sl][:, None])
                        bias[nm].append(t)
                SW[st] = dict(lv=lv, lk=lk, lqr=lqr, **bias)

            # chunk geometry
            chunks = []
            for ch in range(NCH):
                r0 = ch * RCH
                chunks.append((r0, min(RCH, H - r0)))
            pairs = []
            for pc in range(NPAIR):
                a = chunks[2 * pc]
                b = chunks[2 * pc + 1] if 2 * pc + 1 < NCH else None
                pairs.append((a[0], a[1] + (b[1] if b else 0), a, b))
            # accum groups of 20 rows, emitted after the stt pair that completes them

            qchunks = []
            o = 0
            while o < HW:
                qchunks.append((o, min(QF, HW - o)))
                o += QF

            def newP(img, h, stname):
                t = Pp.tile([128, TS], bf16, tag=f"P{h}", name=f"P{stname}{img}{h}")
                nc.gpsimd.memset(t[:, 0:D0], 0.0)
                nc.gpsimd.memset(t[:, D0 + HW : TS], 0.0)
                return t

            def dview(t, o, n):
                return t[:, D0 + o : D0 + o + n]

            # ---------------- preload (cast DMA straight into P) ----------------
            Pw = []
            for img in range(n_img):
                row = []
                for h in range(2):
                    t = newP(img, h, "W")
                    for lc in range(4):
                        o = lc * (HW // 4)
                        with lp():
                            nc.gpsimd.dma_start(
                                out=dview(t, o, HW // 4),
                                in_=xd[img, h * 128 : (h + 1) * 128, o : o + HW // 4],
                            )
                    row.append(t)
                Pw.append(row)

            # ---------------- phases ----------------
            def phase_A(st, sw, cur, e_t, img, gate=None):
                    e_t[img] = bigp.tile([128, HW], bf16, tag="big", name=f"e{st}{img}")
                    for (o, n) in qchunks:
                        pq = psq.tile([128, QF], f32, tag="psq", name="psq")
                        for kt in range(2):
                            nc.tensor.matmul(
                                pq[:, :n],
                                sw["lqr"][kt][:],
                                dview(cur[img][kt], o, n),
                                start=(kt == 0),
                                stop=(kt == 1),
                            )
                        with lp():
                            nc.scalar.activation(
                                e_t[img][:, o : o + n], pq[:, :n], Act.Exp,
                                bias=(gate[:] if gate is not None else 0.0), scale=1.0,
                            )

            def red_A(st, dst, src, nmtag):
                    s1 = scrp.tile([128, HW // 2], bf16, tag="s1", name=f"s1{nmtag}")
                    s2 = src[:, 0 : HW // 4]  # src is fully consumed by fold1
                    v3 = src.rearrange("p (h w) -> p h w", w=W)
                    if st == "W":
                        a0, a1 = v3[:, :, 0:40], v3[:, :, 40:80]
                        w1 = 40
                    else:
                        a0, a1 = v3[:, 0:40, :], v3[:, 40:80, :]
                        w1 = 80
                    sv1 = s1.rearrange("p (h w) -> p h w", w=w1)
                    with lp():
                        nc.vector.tensor_add(sv1[:], a0, a1)
                    if st == "W":
                        b0, b1 = sv1[:, :, 0:20], sv1[:, :, 20:40]
                        w2 = 20
                    else:
                        b0, b1 = sv1[:, 0:20, :], sv1[:, 20:40, :]
                        w2 = 80
                    sv2 = s2.rearrange("p (h w) -> p h w", w=w2)
                    with lp():
                        nc.vector.tensor_add(sv2[:], b0, b1)
                    rsrc = sv2 if st == "W" else s2.rearrange("p (h w) -> p w h", w=w2)
                    nc.vector.reduce_sum(dst[:], rsrc, axis=X)

            def phase_Bd(st, sw, cur, e_t, usd, img):
                    u = [
                        sml.tile([128, H], bf16, tag=f"u{kt}", name=f"u{st}{img}{kt}")
                        for kt in range(2)
                    ]
                    Zt = sml.tile([128, H], f32, tag="Z", name=f"Z{st}{img}")
                    zr = sml.tile([128, H], f32, tag="zr", name=f"zr{st}{img}")
                    for kt in range(2):
                        pr = bigp.tile([128, HW], bf16, tag="big", name=f"pr{st}{img}{kt}")
                        with lp():
                            nc.vector.tensor_mul(
                                pr[:], dview(cur[img][kt], 0, HW), e_t[img][:]
                            )
                        with lp():
                            red_A(st, u[kt], pr[:], f"u{img}{kt}")
                    red_A(st, Zt, e_t[img], f"z{img}")
                    nc.vector.reciprocal(zr[:], Zt[:])
                    usd[img] = (u, zr)

            def phase_ctx(st, sw, usd, ctx2, img):
                    u, zr = usd[img]
                    ctx2[img] = []
                    for mt in range(2):
                        pc = psq.tile([128, QF], f32, tag="psq", name="psctx")
                        for kt in range(2):
                            nc.tensor.matmul(
                                pc[:, :H],
                                sw["lk"][kt][:, mt * 128 : (mt + 1) * 128],
                                u[kt][:],
                                start=(kt == 0),
                                stop=(kt == 1),
                            )
                        cr = sml.tile([128, H], f32, tag=f"cxr{mt}", name=f"cxr{st}{img}{mt}")
                        nc.scalar.activation(
                            cr[:], pc[:, :H], Act.Identity, bias=0.0, scale=0.5
                        )
                        # ctx2 = 0.5*ctx_raw*zr + 0.5*bk  (zr replicated on partitions)
                        ct = sml.tile([128, H], bf16, tag=f"ctx{mt}", name=f"ctx{st}{img}{mt}")
                        with lp():
                            nc.vector.scalar_tensor_tensor(
                                ct[:], in0=cr[:], scalar=1.0, in1=zr[:],
                                op0=Alu.mult, op1=Alu.mult,
                            )
                        with lp():
                            nc.vector.tensor_scalar_add(
                                ct[:], in0=ct[:], scalar1=sw["bk2"][mt][:]
                            )
                        ctx2[img].append(ct)

            def phase_C(st, sw, cur, ctx2, t2d, img, mt):
                    t2t = bigp.tile([128, HW], bf16, tag="big", name=f"t2{st}{img}{mt}")
                    t2d[(img, mt)] = t2t
                    for pi, (r0p, nrt, ca, cb) in enumerate(pairs):
                        th = thp.tile([128, 2 * RCH * W], bf16, tag="th", name="th")
                        for si, cc in enumerate((ca, cb)):
                            if cc is None:
                                continue
                            r0, nr = cc
                            nn = nr * W
                            pv = psv.tile([128, 512], f32, tag="psv", name="psv")
                            for kt in range(2):
                                nc.tensor.matmul(
                                    pv[:, :nn],
                                    sw["lv"][kt][:, mt * 128 : (mt + 1) * 128],
                                    dview(cur[img][kt], r0 * W, nn),
                                    start=(kt == 0),
                                    stop=(kt == 1),
                                )
                            with lp():
                                nc.scalar.activation(
                                    th[:, si * RCH * W : si * RCH * W + nn],
                                    pv[:, :nn],
                                    Act.Tanh, bias=sw["bv2"][mt][:], scale=0.5,
                                )
                        t2v = t2t[:, r0p * W : (r0p + nrt) * W].rearrange(
                            "p (h w) -> p h w", w=W
                        )
                        if st == "W":
                            cb_ap = ctx2[img][mt][:, r0p : r0p + nrt][
                                :, :, None
                            ].broadcast_to([128, nrt, W])
                        else:
                            cb_ap = ctx2[img][mt][:, None, :].broadcast_to([128, nrt, W])
                        with lp():
                            nc.vector.scalar_tensor_tensor(
                                t2v,
                                in0=th[:, : nrt * W].rearrange("p (h w) -> p h w", w=W),
                                scalar=1.0,
                                in1=cb_ap,
                                op0=Alu.add,
                                op1=Alu.mult,
                            )

            def phase_D(st, sw, cur, nxt, img, mt):
                    Pt = cur[img][mt]
                    use8 = st in FP8_STAGES
                    if use8:
                        P8 = P8p.tile([128, TS], f8, tag=f"P8{mt}", name=f"P8{st}{img}{mt}")
                        nc.gpsimd.memset(P8[:, 0:D0], 0.0)
                        nc.gpsimd.memset(P8[:, D0 + HW : TS], 0.0)
                        for cc in range(4):
                            o = cc * (HW // 4)
                            if cc % 2 == 0:
                                with lp():
                                    nc.scalar.activation(
                                        dview(P8, o, HW // 4), dview(Pt, o, HW // 4),
                                        Act.Identity, bias=0.0, scale=1.0,
                                    )
                            else:
                                with lp():
                                    nc.gpsimd.tensor_copy(
                                        out=dview(P8, o, HW // 4), in_=dview(Pt, o, HW // 4)
                                    )
                        Rt = P8
                    else:
                        Rt = Pt
                    ap0 = Rt[:, 0:1]
                    pstride = ap0.ap[0][0]
                    ys_pair = None
                    for ci, (r0, nr) in enumerate(chunks):
                        F = nr * W
                        pd = psdw.tile([128, QF], f32, tag="psdw", name="psdw")
                        pdap = pd[:, 0:1]
                        pd_pstride = pdap.ap[0][0]
                        for ti, (dy, dx) in enumerate(TAPS):
                            off = D0 + (r0 + dy) * W + dx
                            if use8:
                                rhs = bass.AP(ap0.tensor, off, [[pstride, 128], [0, 2], [1, F]])
                                nc.tensor.matmul(
                                    pd[:, :F],
                                    sw["dws"][mt][ti].rearrange("p (two m) -> p two m", two=2),
                                    rhs,
                                    start=(ti == 0), stop=False,
                                    perf_mode=mybir.MatmulPerfMode.DoubleRow,
                                )
                            else:
                                rhs = bass.AP(ap0.tensor, off, [[pstride, 128], [1, F]])
                                nc.tensor.matmul(
                                    pd[:, :F], sw["dws"][mt][ti][:], rhs,
                                    start=(ti == 0), stop=False,
                                )
                        for fi, (dy, dx) in enumerate(FIXUPS):
                            if dx == -1:
                                roff = D0 - 1 + (r0 + dy) * W
                                poff = 0
                            else:
                                roff = D0 + (r0 + dy + 1) * W
                                poff = W - 1
                            pout = bass.AP(
                                pdap.tensor, pdap.offset + poff,
                                [[pd_pstride, 128], [W, nr]],
                            )
                            if use8:
                                rhs = bass.AP(ap0.tensor, roff, [[pstride, 128], [0, 2], [W, nr]])
                                nc.tensor.matmul(
                                    pout,
                                    sw["dwsn"][mt][fi].rearrange("p (two m) -> p two m", two=2),
                                    rhs,
                                    start=False, stop=(fi == 5),
                                    perf_mode=mybir.MatmulPerfMode.DoubleRow,
                                )
                            else:
                                rhs = bass.AP(ap0.tensor, roff, [[pstride, 128], [W, nr]])
                                nc.tensor.matmul(
                                    pout, sw["dwsn"][mt][fi][:], rhs,
                                    start=False, stop=(fi == 5),
                                )
                        if ci % 2 == 0:
                            ys_pair = ysp.tile([128, 2 * RCH * W], bf16, tag="ys", name="ys")
                        with lp():
                            nc.scalar.activation(
                                ys_pair[:, (ci % 2) * RCH * W : (ci % 2) * RCH * W + F],
                                pd[:, :F], Act.Silu,
                                bias=sw["bnsh"][mt][:], scale=sw["bns"][mt][:],
                            )
                        if ci % 2 == 1 or ci == NCH - 1:
                            pc_idx = ci // 2
                            r0p, nrt, ca, cb2 = pairs[pc_idx]
                            nn = nrt * W
                            if st == "W":
                                with lp():
                                    nc.vector.tensor_add(
                                        dview(nxt[img][mt], r0p * W, nn),
                                        dview(Pt, r0p * W, nn),
                                        ys_pair[:, :nn],
                                    )
                            else:
                                for (sr0, snr) in ((ca, cb2) if cb2 else (ca,)):
                                    sn = snr * W
                                    oc = ochp.tile([128, RCH * W], f32, tag="och", name="och")
                                    nc.vector.tensor_add(
                                        oc[:, :sn],
                                        dview(Pt, sr0 * W, sn),
                                        ys_pair[:, (sr0 - r0p) * W : (sr0 - r0p) * W + sn],
                                    )
                                    nc.sync.dma_start(
                                        out=outd[
                                            img, mt * 128 : (mt + 1) * 128,
                                            sr0 * W : sr0 * W + sn,
                                        ],
                                        in_=oc[:, :sn],
                                    )

            # ---------------- stages (software-pipelined W->H) ----------------
            cur = Pw
            SWx = {}
            for st in ("W", "H"):
                sw = SW[st]
                dws, dwsn = [], []
                use8 = st in FP8_STAGES
                for mt in range(2):
                    row = []
                    for ti in range(9):
                        if use8:
                            t = wgt.tile([128, 256], f8, tag=f"dw8{mt}{ti}", name=f"dw8{st}{mt}{ti}")
                            nc.sync.dma_start(out=t[:], in_=prm[st]["dws8"][mt, ti])
                        else:
                            t = wgt.tile([128, 128], bf16, tag=f"dws{mt}{ti}", name=f"dws{st}{mt}{ti}")
                            nc.sync.dma_start(out=t[:], in_=prm[st]["dws"][mt, ti])
                        row.append(t)
                    dws.append(row)
                    rown = []
                    for ti in range(6):
                        if use8:
                            t = wgt.tile([128, 256], f8, tag=f"dn8{mt}{ti}", name=f"dn8{st}{mt}{ti}")
                            nc.sync.dma_start(out=t[:], in_=prm[st]["dwsn8"][mt, ti])
                        else:
                            t = wgt.tile([128, 128], bf16, tag=f"dwn{mt}{ti}", name=f"dwn{st}{mt}{ti}")
                            nc.sync.dma_start(out=t[:], in_=prm[st]["dwsn"][mt, ti])
                        rown.append(t)
                    dwsn.append(rown)
                SWx[st] = dict(sw, dws=dws, dwsn=dwsn)

            stW = {"e": {}, "us": {}, "ctx": {}, "t2": {}}
            stH = {"e": {}, "us": {}, "ctx": {}, "t2": {}}

            def front(st, sw, curx, S, img):
                phase_A(st, sw, curx, S["e"], img, None)
                phase_Bd(st, sw, curx, S["e"], S["us"], img)
                phase_ctx(st, sw, S["us"], S["ctx"], img)
                for mt in range(2):
                    phase_C(st, sw, curx, S["ctx"], S["t2"], img, mt)
                for mt in range(2):
                    t2t = S["t2"][(img, mt)]
                    for (ga, rows, _a, _b) in pairs:
                        with lp():
                            nc.gpsimd.dma_start(
                                out=dview(curx[img][mt], ga * W, rows * W),
                                in_=t2t[:, ga * W : (ga + rows) * W],
                                accum_op=Alu.add,
                            )

            swW, swH = SWx["W"], SWx["H"]
            front("W", swW, cur, stW, 0)
            front("W", swW, cur, stW, 1)
            nxt = {}
            for img in range(n_img):
                nxt[img] = [newP(img, h, "H") for h in range(2)]
            phase_D("W", swW, cur, nxt, 0, 0)
            phase_D("W", swW, cur, nxt, 0, 1)
            front("H", swH, nxt, stH, 0)
            phase_D("W", swW, cur, nxt, 1, 0)
            phase_D("W", swW, cur, nxt, 1, 1)
            front("H", swH, nxt, stH, 1)
            phase_D("H", swH, nxt, None, 0, 0)
            phase_D("H", swH, nxt, None, 0, 1)
            phase_D("H", swH, nxt, None, 1, 0)
            phase_D("H", swH, nxt, None, 1, 1)

    nc.finalize()
    return nc


def _prep_host(inputs):
    import ml_dtypes

    maps = {}
    for st in ("W", "H"):
        wq = np.ascontiguousarray(inputs[f"qkv_w_{st}"], dtype=np.float32)
        bq = np.ascontiguousarray(inputs[f"qkv_b_{st}"], dtype=np.float32)
        dw = np.ascontiguousarray(inputs[f"dw_{st}"], dtype=np.float32)
        gamma = inputs[f"gamma_{st}"].astype(np.float32)
        beta = inputs[f"beta_{st}"].astype(np.float32)
        mean = inputs[f"mean_{st}"].astype(np.float32)
        var = inputs[f"var_{st}"].astype(np.float32)

        maps[f"wvT_{st}"] = np.ascontiguousarray(wq[1 + C :].T).astype(ml_dtypes.bfloat16)
        maps[f"wkT_{st}"] = np.ascontiguousarray(wq[1 : 1 + C].T).astype(ml_dtypes.bfloat16)
        maps[f"wqrT_{st}"] = np.ascontiguousarray(
            np.repeat(wq[0:1].T, 128, axis=1)
        ).astype(ml_dtypes.bfloat16)
        maps[f"bv2_{st}"] = np.ascontiguousarray(0.5 * bq[1 + C :])
        maps[f"bk2_{st}"] = np.ascontiguousarray(0.5 * bq[1 : 1 + C])
        rstd = 1.0 / np.sqrt(var + BN_EPS)
        maps[f"bns_{st}"] = np.ascontiguousarray(gamma * rstd)
        maps[f"bnsh_{st}"] = np.ascontiguousarray(beta - gamma * mean * rstd)

        w9 = dw.reshape(C, 3, 3)
        dws = np.zeros((2, 9, 128, 128), dtype=np.float32)
        for mt in range(2):
            for ti, (dy, dx) in enumerate(TAPS):
                wv = w9[mt * 128 : (mt + 1) * 128, dy + 1, dx + 1]
                dws[mt, ti, np.arange(128), np.arange(128)] = wv
        maps[f"dws_{st}"] = np.ascontiguousarray(dws).astype(ml_dtypes.bfloat16)
        dwsn = np.zeros((2, 6, 128, 128), dtype=np.float32)
        for mt in range(2):
            for fi, (dy, dx) in enumerate(FIXUPS):
                wv = w9[mt * 128 : (mt + 1) * 128, dy + 1, dx + 1]
                dwsn[mt, fi, np.arange(128), np.arange(128)] = -wv
        maps[f"dwsn_{st}"] = np.ascontiguousarray(dwsn).astype(ml_dtypes.bfloat16)

        def split8(w):
            hi = w.astype(ml_dtypes.float8_e4m3fn).astype(np.float32)
            lo = (w - hi).astype(ml_dtypes.float8_e4m3fn).astype(np.float32)
            return hi, lo

        dws8 = np.zeros((2, 9, 128, 2, 128), dtype=np.float32)
        for mt in range(2):
            for ti, (dy, dx) in enumerate(TAPS):
                wv = w9[mt * 128 : (mt + 1) * 128, dy + 1, dx + 1]
                hi, lo = split8(wv)
                dws8[mt, ti, np.arange(128), 0, np.arange(128)] = hi
                dws8[mt, ti, np.arange(128), 1, np.arange(128)] = lo
        maps[f"dws8_{st}"] = np.ascontiguousarray(
            dws8.reshape(2, 9, 128, 256)
        ).astype(ml_dtypes.float8_e4m3fn)
        dwsn8 = np.zeros((2, 6, 128, 2, 128), dtype=np.float32)
        for mt in range(2):
            for fi, (dy, dx) in enumerate(FIXUPS):
                wv = -w9[mt * 128 : (mt + 1) * 128, dy + 1, dx + 1]
                hi, lo = split8(wv)
                dwsn8[mt, fi, np.arange(128), 0, np.arange(128)] = hi
                dwsn8[mt, fi, np.arange(128), 1, np.arange(128)] = lo
        maps[f"dwsn8_{st}"] = np.ascontiguousarray(
            dwsn8.reshape(2, 6, 128, 256)
        ).astype(ml_dtypes.float8_e4m3fn)
    return maps


def _get_nc():
    if "nc" not in _CACHE:
        _CACHE["nc"] = _build()
    return _CACHE["nc"]


def kernel(**inputs):
    from concourse import bass_utils

    nc = _get_nc()
    x = np.ascontiguousarray(inputs["x"], dtype=np.float32).reshape(B, C, HW)
    wmap = _prep_host(inputs)
    in_maps = []
    for c in range(NCORES):
        m = dict(wmap)
        m["x"] = x[c * BPC : (c + 1) * BPC]
        in_maps.append(m)
    res = bass_utils.run_bass_kernel_spmd(nc, in_maps, list(range(NCORES)))
    out = np.concatenate([res.results[c]["out"] for c in range(NCORES)], axis=0)
    return out.reshape(B, C, H, W)



# revision 27
# speedup vs baseline: 1.1379x; 1.1379x over previous
"""DualAxisAggAttn Trainium2 kernel v4 (8-core data-parallel over batch).

Per axis A in {W, H} (x: [B,C,H,W], O = 1+2C):
  qkv = conv1x1(x); s = softmax(q, axis=A); ctx = sum_A(k*s)
  out = x + sigmoid(v)*ctx; y = SiLU(BN(dw3x3(out))); x' = out + y

v4 strategy per core (2 images, channel halves mt0/mt1 on partitions):
  - P [128, 2*TS] fp16 per image: both channel halves in one tile at stride
    TS (row-padded 80x80 layout with zero guard rows).  P8 [128, 2*TS] fp8
    copies made by gpsimd cast-DMAs (f32->fp16/fp8 loads straight from DRAM
    for stage W; SBUF->SBUF fp16->fp8 otherwise).
  - q and v 1x1-conv matmuls run fp8 DoubleRow: one matmul covers the full
    256-channel contraction (kt halves addressed via the [TS,2] rhs dim) at
    0.5 cycles/row.  e = exp(q-4) on Act (shift keeps fp16 u/Z in range and
    cancels in u/Z).  sigmoid(v+bv) directly on Act (no tanh trick).
  - u = sum_A(P*e) via DVE fold tree (fp16, 2x); Z = sum_A(e) reduced on the
    Pool engine; ctx = (Wk@u)*zr + bk with fp16 matmuls.
  - t2 = sg*ctx: DVE tensor_tensor (2x for stage H broadcast); stage W mt1
    half runs on Pool to balance engines.  P += t2 via gpsimd accum-DMA.
  - dw 3x3 = 9 diag fp8 DoubleRow (hi+lo split) matmuls on flat shifted
    views of P8, with 6 negated-diag edge fixups into the same PSUM group.
  - y = SiLU(BN) on Act; finals out+y on DVE fp16 2x (stage W writes stage
    H's P directly; stage H writes fp16 oc, stored by fp16->f32 cast-DMA).
"""

import numpy as np

B, C, H, W = 16, 256, 80, 80
HW = H * W
NCORES = 8
BPC = B // NCORES
TS = 2 + W * (H + 2)   # 6562
D0 = 1 + W             # 81: flat offset of data row 0
RCH = 6
NCH = (H + RCH - 1) // RCH   # 14
NPAIR = (NCH + 1) // 2       # 7
QF = 512
ESHIFT = -4.0
BN_EPS = 1e-5
TAPS = [(dy, dx) for dy in (-1, 0, 1) for dx in (-1, 0, 1)]
FIXUPS = [(dy, dx) for dy in (-1, 0, 1) for dx in (-1, 1)]

_CACHE = {}


def _build(n_img=BPC):
    import concourse.bass as bass
    import concourse.bacc as bacc
    import concourse.mybir as mybir
    import concourse.tile as tile

    f32 = mybir.dt.float32
    f16 = mybir.dt.float16
    f8 = mybir.dt.float8e4
    Alu = mybir.AluOpType
    Act = mybir.ActivationFunctionType
    X = mybir.AxisListType.X
    DR = mybir.MatmulPerfMode.DoubleRow

    nc = bacc.Bacc("TRN2", target_bir_lowering=False, debug=False)

    xd = nc.declare_dram_parameter("x", [n_img, C, HW], f32, isOutput=False)
    prm = {}
    for st in ("W", "H"):
        prm[st] = {
            "wq8": nc.declare_dram_parameter(f"wq8_{st}", [128, 256], f8, isOutput=False),
            "wv8": nc.declare_dram_parameter(f"wv8_{st}", [2, 128, 256], f8, isOutput=False),
            "wkT": nc.declare_dram_parameter(f"wkT_{st}", [C, C], f16, isOutput=False),
            "bv": nc.declare_dram_parameter(f"bv_{st}", [C], f32, isOutput=False),
            "bk": nc.declare_dram_parameter(f"bk_{st}", [C], f32, isOutput=False),
            "bns": nc.declare_dram_parameter(f"bns_{st}", [C], f32, isOutput=False),
            "bnsh": nc.declare_dram_parameter(f"bnsh_{st}", [C], f32, isOutput=False),
            "dws8": nc.declare_dram_parameter(f"dws8_{st}", [2, 9, 128, 256], f8, isOutput=False),
            "dwsn8": nc.declare_dram_parameter(f"dwsn8_{st}", [2, 6, 128, 256], f8, isOutput=False),
        }
    outd = nc.declare_dram_parameter("out", [n_img, C, HW], f32, isOutput=True)

    with tile.TileContext(nc) as tc:
        with (
            tc.tile_pool(name="wgt", bufs=1) as wgt,
            tc.tile_pool(name="P", bufs=3) as Pp,
            tc.tile_pool(name="P8", bufs=2) as P8p,
            tc.tile_pool(name="e", bufs=2) as ep,
            tc.tile_pool(name="pr", bufs=1) as prp,
            tc.tile_pool(name="sg", bufs=1) as sgp,
            tc.tile_pool(name="scr", bufs=1) as scrp,
            tc.tile_pool(name="ys", bufs=2) as ysp,
            tc.tile_pool(name="sml", bufs=1) as sml,
            tc.tile_pool(name="psb", bufs=8, space="PSUM") as psb,
        ):
            lp = lambda: nc.allow_low_precision(reason="f16/f8 within tolerance")

            eshift = wgt.tile([128, 1], f32, tag="eshift", name="eshift")
            nc.gpsimd.memset(eshift[:], ESHIFT)

            # ---------------- weights ----------------
            SW = {}
            for st in ("W", "H"):
                p = prm[st]
                wq8 = wgt.tile([128, 256], f8, tag=f"wq8{st}", name=f"wq8{st}")
                nc.sync.dma_start(out=wq8[:], in_=p["wq8"][:, :])
                wv8 = []
                for mt in range(2):
                    t = wgt.tile([128, 256], f8, tag=f"wv8{st}{mt}", name=f"wv8{st}{mt}")
                    nc.sync.dma_start(out=t[:], in_=p["wv8"][mt])
                    wv8.append(t)
                lk = []
                for kt in range(2):
                    t = wgt.tile([128, C], f16, tag=f"lk{st}{kt}", name=f"lk{st}{kt}")
                    nc.sync.dma_start(out=t[:], in_=p["wkT"][kt * 128 : (kt + 1) * 128, :])
                    lk.append(t)
                bias = {}
                for nm in ("bv", "bk", "bns", "bnsh"):
                    bias[nm] = []
                    for mt in range(2):
                        t = wgt.tile([128, 1], f32, tag=f"{nm}{st}{mt}", name=f"{nm}{st}{mt}")
                        nc.sync.dma_start(out=t[:], in_=p[nm][mt * 128 : (mt + 1) * 128][:, None])
                        bias[nm].append(t)
                dws, dwsn = [], []
                for mt in range(2):
                    row = []
                    for ti in range(9):
                        t = wgt.tile([128, 256], f8, tag=f"dw8{st}{mt}{ti}", name=f"dw8{st}{mt}{ti}")
                        nc.sync.dma_start(out=t[:], in_=p["dws8"][mt, ti])
                        row.append(t)
                    dws.append(row)
                    rown = []
                    for ti in range(6):
                        t = wgt.tile([128, 256], f8, tag=f"dn8{st}{mt}{ti}", name=f"dn8{st}{mt}{ti}")
                        nc.sync.dma_start(out=t[:], in_=p["dwsn8"][mt, ti])
                        rown.append(t)
                    dwsn.append(rown)
                SW[st] = dict(wq8=wq8, wv8=wv8, lk=lk, dws=dws, dwsn=dwsn, **bias)

            # chunk geometry
            chunks = []
            for ch in range(NCH):
                r0 = ch * RCH
                chunks.append((r0, min(RCH, H - r0)))
            pairs = []
            for pc in range(NPAIR):
                a = chunks[2 * pc]
                b = chunks[2 * pc + 1] if 2 * pc + 1 < NCH else None
                pairs.append((a[0], a[1] + (b[1] if b else 0), a, b))

            qchunks = []
            o = 0
            while o < HW:
                qchunks.append((o, min(QF, HW - o)))
                o += QF

            def newP(img, stname):
                t = Pp.tile([128, 2 * TS], f16, tag="P", name=f"P{stname}{img}")
                for mt in range(2):
                    nc.vector.memset(t[:, mt * TS : mt * TS + D0], 0.0)
                    nc.vector.memset(t[:, mt * TS + D0 + HW : (mt + 1) * TS], 0.0)
                return t

            def dview(t, mt, o, n):
                return t[:, mt * TS + D0 + o : mt * TS + D0 + o + n]

            def dual_ap(t, o, n):
                # [[pstride,128],[TS,2],[1,n]] AP over both mt halves of t
                ap0 = t[:, 0:1]
                return bass.AP(ap0.tensor, ap0.offset + D0 + o,
                               [[ap0.ap[0][0], 128], [TS, 2], [1, n]])

            # ---------------- preload ----------------
            Pw, P8x = {}, {}
            for img in range(n_img):
                t8 = P8p.tile([128, 2 * TS], f8, tag="P8", name=f"P8xW{img}")
                with lp():
                    nc.gpsimd.dma_start(
                        out=dual_ap(t8, 0, HW),
                        in_=xd[img].rearrange("(two p) f -> p two f", two=2),
                    )
                P8x[("W", img)] = t8
                t = newP(img, "W")
                with lp():
                    nc.gpsimd.dma_start(
                        out=dual_ap(t, 0, HW),
                        in_=xd[img].rearrange("(two p) f -> p two f", two=2),
                    )
                Pw[img] = t

            # ---------------- phases ----------------
            def phase_A(st, sw, img, e_t):
                # q matmul (fp8 DoubleRow over both kt halves) + exp
                p8 = P8x[(st, img)]
                ap0 = p8[:, 0:1]
                pstride = ap0.ap[0][0]
                et = ep.tile([128, HW], f16, tag="e", name=f"e{st}{img}")
                e_t[img] = et
                lq = sw["wq8"][:].rearrange("p (two m) -> p two m", two=2)
                for (o, n) in qchunks:
                    pq = psb.tile([128, QF], f32, tag="psb", name="psq")
                    nc.tensor.matmul(
                        pq[:, :n],
                        lq,
                        bass.AP(ap0.tensor, ap0.offset + D0 + o,
                                [[pstride, 128], [TS, 2], [1, n]]),
                        start=True, stop=True, perf_mode=DR,
                    )
                    with lp():
                        nc.scalar.activation(
                            et[:, o : o + n], pq[:, :n], Act.Exp,
                            bias=eshift[:], scale=1.0,
                        )

            def phase_Z(st, img, e_t, zrd):
                # Z = sum_A e : fold tree on Pool + row reduce on DVE
                et = e_t[img]
                s1 = scrp.tile([128, HW // 2], f16, tag="s1z", name=f"s1z{st}{img}")
                s2 = scrp.tile([128, HW // 4], f16, tag="s2z", name=f"s2z{st}{img}")
                v3 = et[:].rearrange("p (h w) -> p h w", w=W)
                if st == "W":
                    a0, a1, w1 = v3[:, :, 0:40], v3[:, :, 40:80], 40
                else:
                    a0, a1, w1 = v3[:, 0:40, :], v3[:, 40:80, :], 80
                sv1 = s1.rearrange("p (h w) -> p h w", w=w1)
                with lp():
                    nc.gpsimd.tensor_add(sv1[:], a0, a1)
                if st == "W":
                    b0, b1, w2 = sv1[:, :, 0:20], sv1[:, :, 20:40], 20
                else:
                    b0, b1, w2 = sv1[:, 0:20, :], sv1[:, 20:40, :], 80
                sv2 = s2.rearrange("p (h w) -> p h w", w=w2)
                with lp():
                    nc.vector.tensor_add(sv2[:], b0, b1)
                rsrc = sv2 if st == "W" else s2.rearrange("p (h w) -> p w h", w=w2)
                Zt = sml.tile([128, H], f32, tag="Z", name=f"Z{st}{img}")
                with lp():
                    nc.vector.reduce_sum(Zt[:], rsrc, axis=X)
                zr = sml.tile([128, H], f32, tag="zr", name=f"zr{st}{img}")
                nc.vector.reciprocal(zr[:], Zt[:])
                zrd[img] = zr

            def phase_B(st, sw, img, e_t, ud):
                # u_kt = sum_A (P_kt * e) : DVE mul + fold tree + row reduce
                et = e_t[img]
                u = []
                for kt in range(2):
                    pr = prp.tile([128, HW], f16, tag="pr", name=f"pr{st}{img}{kt}")
                    with lp():
                        nc.vector.tensor_mul(pr[:], dview(Pw[img], kt, 0, HW), et[:])
                    s1 = scrp.tile([128, HW // 2], f16, tag="s1", name=f"s1{st}{img}{kt}")
                    s2 = pr[:, 0 : HW // 4]  # pr fully consumed by fold1
                    v3 = pr[:].rearrange("p (h w) -> p h w", w=W)
                    if st == "W":
                        a0, a1 = v3[:, :, 0:40], v3[:, :, 40:80]
                        w1 = 40
                    else:
                        a0, a1 = v3[:, 0:40, :], v3[:, 40:80, :]
                        w1 = 80
                    sv1 = s1.rearrange("p (h w) -> p h w", w=w1)
                    with lp():
                        nc.vector.tensor_add(sv1[:], a0, a1)
                    if st == "W":
                        b0, b1 = sv1[:, :, 0:20], sv1[:, :, 20:40]
                        w2 = 20
                    else:
                        b0, b1 = sv1[:, 0:20, :], sv1[:, 20:40, :]
                        w2 = 80
                    sv2 = s2.rearrange("p (h w) -> p h w", w=w2)
                    with lp():
                        nc.vector.tensor_add(sv2[:], b0, b1)
                    rsrc = sv2 if st == "W" else s2.rearrange("p (h w) -> p w h", w=w2)
                    ut = sml.tile([128, H], f16, tag=f"u{kt}", name=f"u{st}{img}{kt}")
                    with lp():
                        nc.vector.reduce_sum(ut[:], rsrc, axis=X)
                    u.append(ut)
                ud[img] = u

            def phase_ctx(st, sw, img, ud, zrd, ctxd):
                u, zr = ud[img], zrd[img]
                ctxd[img] = []
                for mt in range(2):
                    pc = psb.tile([128, QF], f32, tag="psb", name="psctx")
                    for kt in range(2):
                        nc.tensor.matmul(
                            pc[:, :H],
                            sw["lk"][kt][:, mt * 128 : (mt + 1) * 128],
                            u[kt][:],
                            start=(kt == 0), stop=(kt == 1),
                        )
                    ct = sml.tile([128, H], f16, tag=f"ctx{mt}", name=f"ctx{st}{img}{mt}")
                    with lp():
                        nc.vector.scalar_tensor_tensor(
                            ct[:], in0=pc[:, :H], scalar=1.0, in1=zr[:],
                            op0=Alu.mult, op1=Alu.mult,
                        )
                    with lp():
                        nc.vector.tensor_scalar_add(ct[:], in0=ct[:], scalar1=sw["bk"][mt][:])
                    ctxd[img].append(ct)

            def phase_Cv(st, sw, img, sgd):
                # v (fp8 DoubleRow) + sigmoid (no ctx dependency)
                p8 = P8x[(st, img)]
                ap0 = p8[:, 0:1]
                pstride = ap0.ap[0][0]
                sgd[img] = []
                for mt in range(2):
                    lv = sw["wv8"][mt][:].rearrange("p (two m) -> p two m", two=2)
                    sgt = sgp.tile([128, HW], f16, tag=f"sg{mt}", name=f"sg{st}{img}{mt}")
                    for (o, n) in qchunks:
                        pv = psb.tile([128, QF], f32, tag="psb", name="psv")
                        nc.tensor.matmul(
                            pv[:, :n],
                            lv,
                            bass.AP(ap0.tensor, ap0.offset + D0 + o,
                                    [[pstride, 128], [TS, 2], [1, n]]),
                            start=True, stop=True, perf_mode=DR,
                        )
                        with lp():
                            nc.scalar.activation(
                                sgt[:, o : o + n], pv[:, :n], Act.Sigmoid,
                                bias=sw["bv"][mt][:], scale=1.0,
                            )
                    sgd[img].append(sgt)

            def phase_t2(st, img, ctxd, sgd):
                # t2 = sg*ctx computed in place on sg, then P += t2 (accum-DMA)
                for mt in range(2):
                    sgt = sgd[img][mt]
                    ct = ctxd[img][mt]
                    for hf in range(4):
                        sv = sgt[:, hf * (HW // 4) : (hf + 1) * (HW // 4)].rearrange(
                            "p (h w) -> p h w", w=W)
                        if st == "W":
                            cb = ct[:, hf * 20 : hf * 20 + 20][:, :, None].broadcast_to(
                                [128, 20, W])
                            eng = nc.vector if (mt == 0 or hf >= 2) else nc.gpsimd
                        else:
                            cb = ct[:, None, :].broadcast_to([128, 20, W])
                            eng = nc.vector
                        with lp():
                            eng.tensor_mul(sv, sv, cb)
                        with lp():
                            nc.gpsimd.dma_start(
                                out=dview(Pw[img], mt, hf * (HW // 4), HW // 4),
                                in_=sgt[:, hf * (HW // 4) : (hf + 1) * (HW // 4)],
                                accum_op=Alu.add,
                            )

            def cast_out(st, img, mt):
                # overwrite P8x[mt] in place with fp8(P[mt]) for the dw conv
                # (q/v readers of the x-fp8 copy are done by now; pads zero)
                t8 = P8x[(st, img)]
                P8x[(st + "o", img)] = t8
                with lp():
                    nc.gpsimd.dma_start(out=t8[:, mt * TS : (mt + 1) * TS],
                                        in_=Pw[img][:, mt * TS : (mt + 1) * TS])

            def phase_D(st, sw, img, mt, nxt):
                P8o = P8x[(st + "o", img)]
                Pt = Pw[img]
                ap0 = P8o[:, 0:1]
                pstride = ap0.ap[0][0]
                base = mt * TS
                ys_pair = None
                pd = None
                pdap = None
                pd_pstride = None
                oc = None
                if st == "H":
                    oc = sgp.tile([128, HW], f16, tag=f"sg{mt}", name=f"oc{img}{mt}")
                for ci, (r0, nr) in enumerate(chunks):
                    F = nr * W
                    hb = ci % 2
                    pd = psb.tile([128, QF], f32, tag="psb", name="psdw")
                    pdap = pd[:, 0:1]
                    pd_pstride = pdap.ap[0][0]
                    if hb == 0:
                        ys_pair = ysp.tile([128, 2 * RCH * W], f16, tag="ys", name="ys")
                    for ti, (dy, dx) in enumerate(TAPS):
                        off = base + D0 + (r0 + dy) * W + dx
                        rhs = bass.AP(ap0.tensor, ap0.offset + off,
                                      [[pstride, 128], [0, 2], [1, F]])
                        nc.tensor.matmul(
                            pd[:, :F],
                            sw["dws"][mt][ti][:].rearrange("p (two m) -> p two m", two=2),
                            rhs,
                            start=(ti == 0), stop=False, perf_mode=DR,
                        )
                    for fi, (dy, dx) in enumerate(FIXUPS):
                        if dx == -1:
                            roff = base + D0 - 1 + (r0 + dy) * W
                            poff = 0
                        else:
                            roff = base + D0 + (r0 + dy + 1) * W
                            poff = W - 1
                        pout = bass.AP(
                            pdap.tensor, pdap.offset + poff,
                            [[pd_pstride, 128], [W, nr]],
                        )
                        rhs = bass.AP(ap0.tensor, ap0.offset + roff,
                                      [[pstride, 128], [0, 2], [W, nr]])
                        nc.tensor.matmul(
                            pout,
                            sw["dwsn"][mt][fi][:].rearrange("p (two m) -> p two m", two=2),
                            rhs,
                            start=False, stop=(fi == 5), perf_mode=DR,
                        )
                    with lp():
                        nc.scalar.activation(
                            ys_pair[:, hb * RCH * W : hb * RCH * W + F],
                            pd[:, :F], Act.Silu,
                            bias=sw["bnsh"][mt][:], scale=sw["bns"][mt][:],
                        )
                    if hb == 1 or ci == NCH - 1:
                        pc_idx = ci // 2
                        r0p, nrt, _a, _b = pairs[pc_idx]
                        nn = nrt * W
                        if st == "W":
                            with lp():
                                nc.vector.tensor_add(
                                    dview(nxt[img], mt, r0p * W, nn),
                                    dview(Pt, mt, r0p * W, nn),
                                    ys_pair[:, :nn],
                                )
                        else:
                            with lp():
                                nc.vector.tensor_add(
                                    oc[:, r0p * W : r0p * W + nn],
                                    dview(Pt, mt, r0p * W, nn),
                                    ys_pair[:, :nn],
                                )
                if st == "H":
                    for hf in range(2):
                        with lp():
                            nc.gpsimd.dma_start(
                                out=outd[img, mt * 128 : (mt + 1) * 128,
                                         hf * (HW // 2) : (hf + 1) * (HW // 2)],
                                in_=oc[:, hf * (HW // 2) : (hf + 1) * (HW // 2)],
                            )

            def cast_in(st, img):
                # P8x for stage H: cast the freshly written P_H tile.  Only
                # data regions are needed (q/v matmuls never read pads);
                # quarter-granular to chase the stage-W finals.
                t8 = P8p.tile([128, 2 * TS], f8, tag="P8", name=f"P8x{st}{img}")
                for mt in range(2):
                    with lp():
                        nc.gpsimd.dma_start(out=dview(t8, mt, 0, HW),
                                            in_=dview(Pw[img], mt, 0, HW))
                P8x[(st, img)] = t8

            # ---------------- schedule ----------------
            stW = {"e": {}, "zr": {}, "u": {}, "ctx": {}, "sg": {}}
            stH = {"e": {}, "zr": {}, "u": {}, "ctx": {}, "sg": {}}

            def front_a(st, sw, img, S):
                phase_A(st, sw, img, S["e"])
                phase_B(st, sw, img, S["e"], S["u"])
                phase_Z(st, img, S["e"], S["zr"])
                phase_Cv(st, sw, img, S["sg"])

            def front_b(st, sw, img, S):
                phase_ctx(st, sw, img, S["u"], S["zr"], S["ctx"])
                phase_t2(st, img, S["ctx"], S["sg"])
                cast_out(st, img, 0)
                cast_out(st, img, 1)

            swW, swH = SW["W"], SW["H"]
            nxt = {}
            phase_A("W", swW, 0, stW["e"])
            phase_B("W", swW, 0, stW["e"], stW["u"])
            phase_Z("W", 0, stW["e"], stW["zr"])
            phase_A("W", swW, 1, stW["e"])
            phase_Cv("W", swW, 0, stW["sg"])
            front_b("W", swW, 0, stW)
            phase_B("W", swW, 1, stW["e"], stW["u"])
            phase_Z("W", 1, stW["e"], stW["zr"])
            phase_Cv("W", swW, 1, stW["sg"])
            front_b("W", swW, 1, stW)
            nxt[0] = newP(0, "H")
            phase_D("W", swW, 0, 0, nxt)
            phase_D("W", swW, 0, 1, nxt)
            nxt[1] = newP(1, "H")
            PwW0, PwW1 = Pw[0], Pw[1]
            Pw[0] = nxt[0]
            cast_in("H", 0)
            phase_A("H", swH, 0, stH["e"])
            phase_B("H", swH, 0, stH["e"], stH["u"])
            phase_Z("H", 0, stH["e"], stH["zr"])
            Pw[0] = PwW0
            phase_D("W", swW, 1, 0, nxt)
            phase_D("W", swW, 1, 1, nxt)
            Pw[0], Pw[1] = nxt[0], nxt[1]
            phase_Cv("H", swH, 0, stH["sg"])
            front_b("H", swH, 0, stH)
            cast_in("H", 1)
            phase_A("H", swH, 1, stH["e"])
            phase_B("H", swH, 1, stH["e"], stH["u"])
            phase_Z("H", 1, stH["e"], stH["zr"])
            phase_Cv("H", swH, 1, stH["sg"])
            front_b("H", swH, 1, stH)
            phase_D("H", swH, 0, 0, None)
            phase_D("H", swH, 0, 1, None)
            phase_D("H", swH, 1, 0, None)
            phase_D("H", swH, 1, 1, None)

    nc.finalize()
    return nc


def _prep_host(inputs):
    import ml_dtypes

    f8t = ml_dtypes.float8_e4m3fn
    maps = {}
    for st in ("W", "H"):
        wq = np.ascontiguousarray(inputs[f"qkv_w_{st}"], dtype=np.float32)
        bq = np.ascontiguousarray(inputs[f"qkv_b_{st}"], dtype=np.float32)
        dw = np.ascontiguousarray(inputs[f"dw_{st}"], dtype=np.float32)
        gamma = inputs[f"gamma_{st}"].astype(np.float32)
        beta = inputs[f"beta_{st}"].astype(np.float32)
        mean = inputs[f"mean_{st}"].astype(np.float32)
        var = inputs[f"var_{st}"].astype(np.float32)

        wqv = wq[0]              # [C]
        wkm = wq[1 : 1 + C]      # [C, C]
        wvm = wq[1 + C :]        # [C, C]

        wq8 = np.zeros((128, 2, 128), dtype=np.float32)
        for two in range(2):
            wq8[:, two, :] = wqv[two * 128 : (two + 1) * 128][:, None]
        maps[f"wq8_{st}"] = np.ascontiguousarray(wq8.reshape(128, 256)).astype(f8t)

        wv8 = np.zeros((2, 128, 2, 128), dtype=np.float32)
        for mt in range(2):
            for two in range(2):
                # lhsT[p, two, m] = Wv[mt*128+m, p + two*128]
                wv8[mt, :, two, :] = wvm[mt * 128 : (mt + 1) * 128,
                                         two * 128 : (two + 1) * 128].T
        maps[f"wv8_{st}"] = np.ascontiguousarray(wv8.reshape(2, 128, 256)).astype(f8t)

        maps[f"wkT_{st}"] = np.ascontiguousarray(wkm.T).astype(np.float16)
        maps[f"bv_{st}"] = np.ascontiguousarray(bq[1 + C :])
        maps[f"bk_{st}"] = np.ascontiguousarray(bq[1 : 1 + C])
        rstd = 1.0 / np.sqrt(var + BN_EPS)
        maps[f"bns_{st}"] = np.ascontiguousarray(gamma * rstd)
        maps[f"bnsh_{st}"] = np.ascontiguousarray(beta - gamma * mean * rstd)

        def split8(w):
            hi = w.astype(f8t).astype(np.float32)
            lo = (w - hi).astype(f8t).astype(np.float32)
            return hi, lo

        w9 = dw.reshape(C, 3, 3)
        dws8 = np.zeros((2, 9, 128, 2, 128), dtype=np.float32)
        for mt in range(2):
            for ti, (dy, dx) in enumerate(TAPS):
                wv_ = w9[mt * 128 : (mt + 1) * 128, dy + 1, dx + 1]
                hi, lo = split8(wv_)
                dws8[mt, ti, np.arange(128), 0, np.arange(128)] = hi
                dws8[mt, ti, np.arange(128), 1, np.arange(128)] = lo
        maps[f"dws8_{st}"] = np.ascontiguousarray(dws8.reshape(2, 9, 128, 256)).astype(f8t)
        dwsn8 = np.zeros((2, 6, 128, 2, 128), dtype=np.float32)
        for mt in range(2):
            for fi, (dy, dx) in enumerate(FIXUPS):
                wv_ = -w9[mt * 128 : (mt + 1) * 128, dy + 1, dx + 1]
                hi, lo = split8(wv_)
                dwsn8[mt, fi, np.arange(128), 0, np.arange(128)] = hi
                dwsn8[mt, fi, np.arange(128), 1, np.arange(128)] = lo
        maps[f"dwsn8_{st}"] = np.ascontiguousarray(dwsn8.reshape(2, 6, 128, 256)).astype(f8t)
    return maps


def _get_nc():
    if "nc" not in _CACHE:
        _CACHE["nc"] = _build()
    return _CACHE["nc"]


def kernel(**inputs):
    from concourse import bass_utils

    nc = _get_nc()
    x = np.ascontiguousarray(inputs["x"], dtype=np.float32).reshape(B, C, HW)
    wmap = _prep_host(inputs)
    in_maps = []
    for c in range(NCORES):
        m = dict(wmap)
        m["x"] = x[c * BPC : (c + 1) * BPC]
        in_maps.append(m)
    res = bass_utils.run_bass_kernel_spmd(nc, in_maps, list(range(NCORES)))
    out = np.concatenate([res.results[c]["out"] for c in range(NCORES)], axis=0)
    return out.reshape(B, C, H, W)
